# revision 4
# baseline (speedup 1.0000x reference)
"""Trainium2 Bass kernel for nn_CachedAttention (B=4, T=2048, D=2048, H=16, start_pos=0).

Sharding: 8 cores = 4 batches x 2 head-groups. Core i handles batch i//2 and
heads (i%2)*8 .. (i%2)*8+8. Each core computes QKV projections for its heads,
causal attention, and a partial output projection (its heads' contribution to
the full output). The host sums the two partials per batch.

All matmul operands are bf16 (fp32 PSUM accumulation). Scores are built
transposed (S.T[kpos, qpos]) so the exp'd probabilities feed the PV matmul
directly as the moving operand; softmax denominators come from a ones-vector
matmul (partition-dim reduction on the PE). Fully-masked causal tiles are
skipped; diagonal tiles get a 0/1 mask multiply after exp.
"""
import math

import numpy as np
import ml_dtypes

import concourse.bass as bass
import concourse.tile as tile
from concourse import mybir
from concourse.bass_utils import run_bass_kernel_spmd
from concourse.masks import make_identity
from concourse.vector_clock import ScopedClock

bf16 = mybir.dt.bfloat16
f32 = mybir.dt.float32

B, T, D, H = 4, 2048, 2048, 16
DK = D // H          # 128
HL = H // 2          # heads per core = 8
FT = D // 128        # feature tiles = 16
TT = T // 128        # token tiles = 16
NC_CHUNK = 512       # qpos chunk
NCH = T // NC_CHUNK  # 4 chunks
SCALE = 1.0 / math.sqrt(DK)
N_CORES = 8


# ---------------------------------------------------------------------------
# Workaround: this toolchain's walrus rejects Drain instructions that carry
# attached sem waits ("Too many sync wait commands"). Emit the global-clock
# waits as standalone wait_ge instructions instead, then a bare drain.
# ---------------------------------------------------------------------------
def _patched_drain_and_barrier(self, tick_clock, wait_clock):
    nop = self.nc.sync.nop()
    wait_clock.add_sem_waits(nop.ins, ScopedClock({None: tick_clock.global_clock}))
    si = nop.ins.sync_info
    waits = list(si.on_wait or []) if si is not None else []
    if si is not None and waits:
        si.on_wait = []
    handles = {h.num: h for h in self.sems.allocated().values()}
    for w in waits:
        assert w.wait_mode == "sem-ge-imm", w
        h = handles.get(w.id)
        assert h is not None, f"no handle for sem id {w.id} ({w.ant_name})"
        self.nc.sync.wait_ge(h, w.wait_value)
    self.nc.sync.drain()
    self.nc.all_engine_barrier(sem_only=True)
    assert self.sems is not None
    popped = self.nc._tile_sem_poison_stack.pop()
    assert popped is self._sem_poison
    self.nc.clear_and_free_semaphores(list(self.sems.allocated().values()))
    self.nc.all_engine_barrier(sem_only=True)


def _apply_tile_patch():
    tile.TileContext._drain_and_barrier = _patched_drain_and_barrier


def _hoist_excess_waits(nc, cap=1):
    """Walrus in this toolchain fits at most `cap` attached sem-waits per
    instruction. Hoist extras into standalone InstEventSemaphore waits emitted
    immediately before, on the same engine."""
    import bass_rust
    for f in nc.m.functions:
        for blk in f.blocks:
            new = []
            for inst in blk.instructions:
                si = inst.sync_info
                ow = list(si.on_wait) if si is not None and si.on_wait else []
                if len(ow) > cap:
                    # keep non-ge waits attached first (barrier eq-waits)
                    keep = [w for w in ow if w.wait_mode != "sem-ge-imm"]
                    hoist = [w for w in ow if w.wait_mode == "sem-ge-imm"]
                    while len(keep) < cap and hoist:
                        keep.append(hoist.pop())
                    assert len(keep) <= cap, (inst.name, ow)
                    for k, w in enumerate(hoist):
                        ev = mybir.InstEventSemaphore(
                            name=f"{inst.name}-w{k}",
                            engine=inst.engine,
                            ins=[],
                            outs=[],
                            sync_info=bass_rust.SyncInfo(
                                on_wait=[w], on_update=[]),
                        )
                        nc.register_instruction(ev)
                        new.append(ev)
                    si.on_wait = keep
                new.append(inst)
            blk.instructions = new


# ---------------------------------------------------------------------------
# Device program (identical on all 8 cores; per-core data comes via in_maps)
# ---------------------------------------------------------------------------
def build_program():
    _apply_tile_patch()
    nc = bass.Bass()

    xT_d = nc.dram_tensor("xT", [FT, 128, T], bf16, kind="ExternalInput")
    wq_d = nc.dram_tensor("wq", [HL, 128, FT, DK], bf16, kind="ExternalInput")
    wk_d = nc.dram_tensor("wk", [HL, 128, FT, DK], bf16, kind="ExternalInput")
    wv_d = nc.dram_tensor("wv", [HL, 128, FT, DK], bf16, kind="ExternalInput")
    wo_d = nc.dram_tensor("wo", [HL, 128, D], bf16, kind="ExternalInput")
    cm_d = nc.dram_tensor("cm", [4, 128, NC_CHUNK], bf16, kind="ExternalInput")
    out_d = nc.dram_tensor("out", [T, D], f32, kind="ExternalOutput")
    rb_d = nc.dram_tensor("rb", [HL, NCH, NC_CHUNK], f32)  # recip bounce

    with tile.TileContext(nc) as tc:
        with (
            tc.tile_pool(name="xt", bufs=FT) as xt_pool,
            tc.tile_pool(name="wstrip", bufs=2) as w_pool,
            tc.tile_pool(name="qkv", bufs=2) as qkv_pool,
            tc.tile_pool(name="vh", bufs=2 * TT) as vh_pool,
            tc.tile_pool(name="yt", bufs=HL) as y_pool,
            tc.tile_pool(name="wo", bufs=2 * HL) as wo_pool,
            tc.tile_pool(name="pt", bufs=3) as pt_pool,
            tc.tile_pool(name="small", bufs=4) as small_pool,
            tc.tile_pool(name="const", bufs=1) as const_pool,
            tc.tile_pool(name="ost", bufs=3) as o_pool,
            tc.tile_pool(name="ps_st", bufs=2, space="PSUM") as ps_st,
            tc.tile_pool(name="ps_ot", bufs=2, space="PSUM") as ps_ot,
            tc.tile_pool(name="ps_dn", bufs=1, space="PSUM") as ps_dn,
            tc.tile_pool(name="ps_mm", bufs=3, space="PSUM") as ps_mm,
        ):
            # constants
            ones = const_pool.tile([128, 1], bf16, tag="ones")
            nc.vector.memset(ones[:], 1.0)
            ident = const_pool.tile([128, 128], bf16, tag="ident")
            make_identity(nc, ident[:])
            cm_sb = []
            for j in range(4):
                t_ = const_pool.tile([128, NC_CHUNK], bf16, tag=f"cm{j}")
                nc.sync.dma_start(out=t_[:], in_=cm_d[j])
                cm_sb.append(t_)

            # resident x.T tiles (feature-major), chunked loads for early start
            xt_sb = []
            for f in range(FT):
                t_ = xt_pool.tile([128, T], bf16, tag="xt")
                for c in range(NCH):
                    nc.sync.dma_start(
                        out=t_[:, c * NC_CHUNK:(c + 1) * NC_CHUNK],
                        in_=xT_d[f][:, c * NC_CHUNK:(c + 1) * NC_CHUNK],
                    )
                xt_sb.append(t_)

            yt_sb = []
            for h in range(HL):
                # --- weight strips for this head ---
                wq_s = w_pool.tile([128, FT, DK], bf16, tag="wq")
                nc.sync.dma_start(out=wq_s[:], in_=wq_d[h])
                wk_s = w_pool.tile([128, FT, DK], bf16, tag="wk")
                nc.sync.dma_start(out=wk_s[:], in_=wk_d[h])
                wv_s = w_pool.tile([128, FT, DK], bf16, tag="wv")
                nc.sync.dma_start(out=wv_s[:], in_=wv_d[h])

                # --- q.T / k.T / v.T projections: [dk=128, T] each ---
                def project(w_s, tag):
                    dst = qkv_pool.tile([128, T], bf16, tag=tag)
                    for c in range(NCH):
                        ps = ps_mm.tile([128, NC_CHUNK], f32, tag="mm")
                        for f in range(FT):
                            nc.tensor.matmul(
                                ps[:],
                                w_s[:, f, :],
                                xt_sb[f][:, c * NC_CHUNK:(c + 1) * NC_CHUNK],
                                start=(f == 0),
                                stop=(f == FT - 1),
                            )
                        nc.vector.tensor_copy(
                            out=dst[:, c * NC_CHUNK:(c + 1) * NC_CHUNK], in_=ps[:]
                        )
                    return dst

                qT_s = project(wq_s, "qT")
                kT_s = project(wk_s, "kT")
                vT_s = project(wv_s, "vT")

                # --- v natural layout via PE transpose: 16 tiles [tok128, dk] ---
                vh = []
                for tt in range(TT):
                    pst = ps_mm.tile([128, 128], bf16, tag="mm")
                    nc.tensor.transpose(
                        pst[:], vT_s[:, tt * 128:(tt + 1) * 128], ident[:]
                    )
                    vt = vh_pool.tile([128, 128], bf16, tag="vh")
                    nc.vector.tensor_copy(out=vt[:], in_=pst[:])
                    vh.append(vt)

                # --- attention (causal), chunk of 512 query positions ---
                yT_s = y_pool.tile([128, T], bf16, tag="yt")
                yt_sb.append(yT_s)
                for c in range(NCH):
                    nt = 4 * c + 4  # kpos tiles 0..nt-1 (rest fully masked)
                    ot = ps_ot.tile([128, NC_CHUNK], f32, tag="ot")
                    dn = ps_dn.tile([1, NC_CHUNK], f32, tag="dn")
                    qs = qT_s[:, c * NC_CHUNK:(c + 1) * NC_CHUNK]
                    for t in range(nt):
                        st = ps_st.tile([128, NC_CHUNK], f32, tag="st")
                        nc.tensor.matmul(
                            st[:], kT_s[:, t * 128:(t + 1) * 128], qs,
                            start=True, stop=True,
                        )
                        pt = pt_pool.tile([128, NC_CHUNK], bf16, tag="pt")
                        nc.scalar.activation(
                            out=pt[:], in_=st[:],
                            func=mybir.ActivationFunctionType.Exp, scale=SCALE,
                        )
                        if t >= 4 * c:  # diagonal tile: 0/1 mask after exp
                            nc.vector.tensor_mul(pt[:], pt[:], cm_sb[t - 4 * c][:])
                        nc.tensor.matmul(
                            ot[:], vh[t][:], pt[:],
                            start=(t == 0), stop=(t == nt - 1),
                        )
                        nc.tensor.matmul(
                            dn[:], ones[:], pt[:],
                            start=(t == 0), stop=(t == nt - 1),
                        )
                    # normalize: out_tile = ot / colsum  (broadcast via DRAM bounce)
                    rc = small_pool.tile([1, NC_CHUNK], f32, tag="rc")
                    nc.vector.reciprocal(rc[:], dn[:])
                    nc.sync.dma_start(out=rb_d[h, c], in_=rc[:])
                    bc = small_pool.tile([128, NC_CHUNK], f32, tag="bc")
                    r_ap = rb_d[h, c]
                    bcast = bass.AP(
                        tensor=r_ap.tensor, offset=r_ap.offset,
                        ap=[[0, 128]] + list(r_ap.ap),
                    )
                    nc.sync.dma_start(out=bc[:], in_=bcast)
                    nc.vector.tensor_mul(
                        yT_s[:, c * NC_CHUNK:(c + 1) * NC_CHUNK], ot[:], bc[:]
                    )

            # --- output projection: out[tok, e] += y.T[d, tok]^T @ woT[d, e] ---
            for dc in range(NCH):
                wo_strips = []
                for h in range(HL):
                    ws = wo_pool.tile([128, NC_CHUNK], bf16, tag="wo")
                    nc.sync.dma_start(
                        out=ws[:], in_=wo_d[h][:, dc * NC_CHUNK:(dc + 1) * NC_CHUNK]
                    )
                    wo_strips.append(ws)
                for tt in range(TT):
                    po = ps_mm.tile([128, NC_CHUNK], f32, tag="mm")
                    for h in range(HL):
                        nc.tensor.matmul(
                            po[:],
                            yt_sb[h][:, tt * 128:(tt + 1) * 128],
                            wo_strips[h][:],
                            start=(h == 0),
                            stop=(h == HL - 1),
                        )
                    o_s = o_pool.tile([128, NC_CHUNK], f32, tag="o")
                    nc.vector.tensor_copy(out=o_s[:], in_=po[:])
                    nc.sync.dma_start(
                        out=out_d[tt * 128:(tt + 1) * 128,
                                  dc * NC_CHUNK:(dc + 1) * NC_CHUNK],
                        in_=o_s[:],
                    )

    _hoist_excess_waits(nc)
    nc.finalize()
    return nc


_NC_CACHE = None


def get_program():
    global _NC_CACHE
    if _NC_CACHE is None:
        _NC_CACHE = build_program()
    return _NC_CACHE


# ---------------------------------------------------------------------------
# Host-side sharding / layout prep
# ---------------------------------------------------------------------------
def _causal_masks():
    cm = np.zeros((4, 128, NC_CHUNK), dtype=np.float32)
    kp = np.arange(128)[:, None]
    qp = np.arange(NC_CHUNK)[None, :]
    for j in range(4):
        cm[j] = (128 * j + kp <= qp).astype(np.float32)
    return cm.astype(ml_dtypes.bfloat16)


def _w_strips(w_loc):
    # w_loc: [1024 out-dims, 2048 feat] -> [h, p(feat within tile), f, j(dk)]
    a = np.ascontiguousarray(w_loc.reshape(HL, DK, FT, 128).transpose(0, 3, 2, 1))
    return a.astype(ml_dtypes.bfloat16)


def make_in_maps(x, Wq, Wk, Wv, Wo):
    cm = _causal_masks()
    in_maps = []
    for core in range(N_CORES):
        b, hg = core // 2, core % 2
        sl = slice(hg * HL * DK, (hg + 1) * HL * DK)
        xT = np.ascontiguousarray(x[b].T).reshape(FT, 128, T)
        wo_loc = np.ascontiguousarray(Wo[:, sl].T).reshape(HL, 128, D)
        in_maps.append({
            "xT": xT.astype(ml_dtypes.bfloat16),
            "wq": _w_strips(Wq[sl, :]),
            "wk": _w_strips(Wk[sl, :]),
            "wv": _w_strips(Wv[sl, :]),
            "wo": wo_loc.astype(ml_dtypes.bfloat16),
            "cm": cm,
        })
    return in_maps


def combine(results):
    out = np.empty((B, T, D), dtype=np.float32)
    for b in range(B):
        out[b] = results[2 * b]["out"] + results[2 * b + 1]["out"]
    return out


def kernel(x, Wq, Wk, Wv, Wo, k_cache, v_cache, start_pos, **_ignored):
    x = np.asarray(x, dtype=np.float32)
    Wq = np.asarray(Wq, dtype=np.float32)
    Wk = np.asarray(Wk, dtype=np.float32)
    Wv = np.asarray(Wv, dtype=np.float32)
    Wo = np.asarray(Wo, dtype=np.float32)
    assert int(start_pos) == 0, "kernel specialized for start_pos=0 prefill"
    assert x.shape == (B, T, D)

    nc = get_program()
    in_maps = make_in_maps(x, Wq, Wk, Wv, Wo)
    res = run_bass_kernel_spmd(nc, in_maps, list(range(N_CORES)))
    return combine(res.results)


if __name__ == "__main__":
    rng = np.random.default_rng(0)
    x = rng.standard_normal((B, T, D)).astype(np.float32)
    mk = lambda: (rng.standard_normal((D, D)) * 0.02).astype(np.float32)
    out = kernel(x, mk(), mk(), mk(), mk(),
                 np.zeros((B, H, T, DK), np.float32),
                 np.zeros((B, H, T, DK), np.float32), 0)
    print(out.shape, out.dtype, np.abs(out).max())
